# revision 15
# baseline (speedup 1.0000x reference)
"""CTC loss on 8 Trainium2 cores.

Strategy (data-parallel over batch, B=64 -> 8 utterances/core):
  Device per core, two concurrent pipelines:
    - Streaming (ACT + HWDGE-DMA): acts [3200, 5000] f32 in 13 big tiles
      [128, 2, 5000]; ScalarE exp with accum_out -> Z per (t,u) row.
      ~64MB/core, memory-bound.
    - CTC DP on DVE only, layout [8 utterances (partitions), 101 ext states
      (free)]: shifts are free-axis offset views with zero guard columns;
      per step 3 DVE ops (add, add, dup-mul via broadcast+2-block write)
      in bf16 (2x DVE mode). Skip mask folded into a second emission table
      q1 = q0*mask; emissions q = exp(gtilde) precomputed host-side in
      bf16, streamed in chunks (the first four, small, ride the ACT HWDGE
      ring ahead of the acts tiles for a fast DP start; the rest use the
      sync-engine ring). Rescale by sum every 32 steps via
      scalar_tensor_tensor with a per-partition reciprocal; the STT accum
      also logs the rescale constants c into cs.
    - Freeze (t >= input_len) and final readout folded into one-hot q
      columns; final step's STT accum gives the end-state mass directly.
  Device outputs are raw Z [128, 25] and cs [8, 13]; the host does the
  tiny ln + masked reductions, corrections sum(gmax) - sum(logZ), mean.
  (tensor_tensor_reduce is avoided: it crashes this runtime; fp16 state
  underflows CTC alpha — bf16 is required.)
"""
import numpy as np
import ml_dtypes

import concourse.bass as bass
import concourse.bacc as bacc
import concourse.mybir as mybir
import concourse.tile as tile
from concourse.bass_utils import run_bass_kernel_spmd

T, B, V, L = 400, 64, 5000, 50
S = 2 * L + 1            # 101
NCORES = 8
BS = B // NCORES         # 8
ROWS = T * BS            # 3200
NT = 12                  # full [128, 2, V] tiles; + one [128, V] tail tile
NZ = 2 * NT + 1          # 25 Z slots
BOOST = np.float32(1.5)
K_RES = 32
NRES = T // K_RES        # 12 rescales; cs has NRES+1 slots
QW = 2 * S               # 202
QCOLS = (T + 1) * QW     # 81002
# q chunk boundaries in t: small first chunks for a fast DP start
CHB = [0, 12, 22, 37, 57, 87, 127, 177, 227, 277, 327, 377, T + 1]
CHMAX = max(b - a for a, b in zip(CHB, CHB[1:]))
NEG = np.float32(-10000.0)
F32 = mybir.dt.float32
BF16 = mybir.dt.bfloat16
bf16 = ml_dtypes.bfloat16
AF = mybir.ActivationFunctionType
ALU = mybir.AluOpType


def _build_program():
    nc = bacc.Bacc(None, target_bir_lowering=False)
    acts = nc.dram_tensor("acts", [ROWS, V], F32, kind="ExternalInput")
    qq = nc.dram_tensor("qq", [BS, QCOLS], BF16, kind="ExternalInput")
    out_z = nc.dram_tensor("out_z", [128, NZ], F32, kind="ExternalOutput")
    out_cs = nc.dram_tensor("out_cs", [BS, NRES + 1], F32,
                            kind="ExternalOutput")

    with tile.TileContext(nc) as tc:
        with (
            tc.tile_pool(name="mp", bufs=1) as mp,
            tc.tile_pool(name="ap", bufs=2) as ap,
            tc.tile_pool(name="qp", bufs=5) as qp,
        ):
            # ---------------- persistent state ----------------
            ws = mp.tile([BS, 2 * S + 4], BF16)   # [g g a(101) g g b(101)]
            nc.vector.memset(ws[:], 0.0)
            t1 = mp.tile([BS, S], BF16)
            t2 = mp.tile([BS, S], BF16)
            cs = mp.tile([BS, NRES + 1], F32)
            rbuf = mp.tile([BS, 1], F32)
            zbuf = mp.tile([128, NZ], F32)

            def wsb():   # state write view: cols {2..102, 105..205}
                return ws[:].rearrange("p (b s) -> p b s", b=2)[:, :, 2:S + 2]

            def t2b():   # t2 broadcast over the two state blocks
                return t2[:].unsqueeze(1).broadcast_to((BS, 2, S))

            # ------------- streaming pipeline (ACT engine + ring) -------
            # First two (small) q chunks ride the ACT HWDGE ring ahead of
            # the 5MB acts tiles, so the DP starts within a few us.
            qts = {}
            for c in range(4):
                qte = qp.tile([BS, CHMAX * QW], BF16, tag="q")
                qts[c] = qte
                nc.scalar.dma_start(
                    qte[:, 0:(CHB[c + 1] - CHB[c]) * QW],
                    qq[:, CHB[c] * QW:CHB[c + 1] * QW])
            for k in range(NT + 1):
                at = ap.tile([128, 2 * V], F32, tag="acts")
                if k < NT:
                    src = acts[256 * k:256 * (k + 1), :].rearrange(
                        "(b p) v -> p b v", b=2)
                    nc.scalar.dma_start(at[:], src)
                    for bb in range(2):
                        blk = at[:, bb * V:(bb + 1) * V]
                        nc.scalar.activation(blk, blk, AF.Exp,
                                             accum_out=zbuf[:, 2 * k + bb:
                                                            2 * k + bb + 1])
                else:
                    nc.scalar.dma_start(at[:, 0:V], acts[256 * NT:ROWS, :])
                    blk = at[:, 0:V]
                    nc.scalar.activation(blk, blk, AF.Exp,
                                         accum_out=zbuf[:, 2 * NT:2 * NT + 1])
            nc.scalar.dma_start(out_z[:], zbuf[:])

            # ------------- DP pipeline (DVE + sync-ring DMA) ------------
            for c in range(len(CHB) - 1):
                t0, tend = CHB[c], CHB[c + 1]
                if c < 4:
                    qt = qts[c]
                else:
                    qt = qp.tile([BS, CHMAX * QW], BF16, tag="q")
                    nc.sync.dma_start(qt[:, 0:(tend - t0) * QW],
                                      qq[:, t0 * QW:tend * QW])
                trange = range(max(t0, 1), tend)
                if c == 0:
                    nc.vector.tensor_copy(
                        wsb(), qt[:, 0:QW].rearrange("p (b s) -> p b s", b=2))
                for t in trange:
                    base = (t - t0) * QW
                    w = min(2 * t + 2, S)   # active-state prefix band
                    q01 = qt[:, base:base + QW].rearrange(
                        "p (b s) -> p b s", b=2)[:, :, 0:w]
                    wsv = wsb()[:, :, 0:w]
                    t2v = t2[:, 0:w].unsqueeze(1).broadcast_to((BS, 2, w))
                    nc.vector.tensor_add(t1[:, 0:w], ws[:, 2:w + 2],
                                         ws[:, 1:w + 1])
                    nc.vector.tensor_add(t2[:, 0:w], t1[:, 0:w],
                                         ws[:, S + 2:S + 2 + w])
                    if t % K_RES == K_RES - 1:
                        j = t // K_RES
                        nc.vector.scalar_tensor_tensor(
                            wsv, t2v, 1.0, q01, ALU.mult, ALU.mult,
                            accum_out=cs[:, j:j + 1])
                        nc.vector.reciprocal(rbuf[:], cs[:, j:j + 1])
                    elif t % K_RES == 0:
                        nc.vector.scalar_tensor_tensor(
                            wsv, t2v, rbuf[:, 0:1], q01, ALU.mult, ALU.mult)
                    elif t == T:
                        nc.vector.scalar_tensor_tensor(
                            wsv, t2v, 1.0, q01, ALU.mult, ALU.mult,
                            accum_out=cs[:, NRES:NRES + 1])
                    else:
                        nc.vector.tensor_mul(wsv, t2v, q01)
            nc.sync.dma_start(out_cs[:], cs[:])
    nc.compile()
    return nc


_PROGRAM = None
_LAST_RESULTS = None


def _get_program():
    global _PROGRAM
    if _PROGRAM is None:
        _PROGRAM = _build_program()
    return _PROGRAM


def _host_prep(acts, ilen, labels, llen):
    """Per-core input maps plus host-side correction sums."""
    ext = np.zeros((B, S), np.int64)
    ext[:, 1::2] = labels
    m = np.zeros((B, S), np.float32)
    m[:, 2:] = ((ext[:, 2:] != 0) & (ext[:, 2:] != ext[:, :-2])).astype(
        np.float32)
    mtil = np.zeros((B, S), np.float32)
    mtil[:, :S - 2] = m[:, 2:]

    g = np.take_along_axis(acts, np.broadcast_to(ext[None], (T, B, S)), axis=2)
    gmax = g.max(axis=2).astype(np.float32) - BOOST          # [T, B]
    gt = (g - gmax[:, :, None]).astype(np.float32)           # [T, B, S]

    srange = np.arange(S)
    valid_s = srange[None, :] < (2 * llen + 1)[:, None]      # [B, S]
    gt = np.where(valid_s[None], gt, NEG)
    onehot = np.where(srange[None, :] == (2 * llen)[:, None],
                      np.float32(0.0), NEG)                  # [B, S]
    tmask = np.arange(T)[:, None] < ilen[None, :]            # [T, B]
    gt = np.where(tmask[:, :, None], gt, onehot[None])
    gt[0, :, 2:] = NEG
    gt_all = np.concatenate([gt, onehot[None]], axis=0)      # [T+1, B, S]

    q0 = np.exp(gt_all, dtype=np.float32)                    # [T+1, B, S]
    q1 = q0 * mtil[None]
    qarr = np.empty((B, T + 1, 2, S), np.float32)
    qarr[:, :, 0, :] = q0.transpose(1, 0, 2)
    qarr[:, :, 1, :] = q1.transpose(1, 0, 2)
    qq_full = qarr.reshape(B, QCOLS).astype(bf16)

    sum_gmax = (gmax.astype(np.float64) * tmask).sum(axis=0)  # [B]

    in_maps = []
    for c in range(NCORES):
        csl = slice(c * BS, (c + 1) * BS)
        acts_c = np.ascontiguousarray(
            acts[:, csl, :].reshape(ROWS, V).astype(np.float32))
        qq_c = np.ascontiguousarray(qq_full[csl])
        in_maps.append({"acts": acts_c, "qq": qq_c})
    return in_maps, sum_gmax


# z slot mapping: out_z[p, c] is Z for acts row r(p, c); t = r//8, u = p%8
_P = np.arange(128)
_RCOLS = np.empty((128, NZ), np.int64)
for _c in range(2 * NT):
    _RCOLS[:, _c] = 256 * (_c // 2) + 128 * (_c % 2) + _P
_RCOLS[:, 2 * NT] = 256 * NT + _P
_TCOLS = _RCOLS // BS          # [128, NZ]
_UCOLS = _P % BS               # [128]


def kernel(activations, input_lengths, labels, label_lengths):
    acts = np.asarray(activations, dtype=np.float32)
    ilen = np.asarray(input_lengths, dtype=np.int64)
    labs = np.asarray(labels, dtype=np.int64)
    llen = np.asarray(label_lengths, dtype=np.int64)

    in_maps, sum_gmax = _host_prep(acts, ilen, labs, llen)
    nc = _get_program()
    _r = run_bass_kernel_spmd(nc, in_maps, list(range(NCORES)))
    global _LAST_RESULTS
    _LAST_RESULTS = _r
    res = _r.results

    losses = np.zeros(B, np.float64)
    for c in range(NCORES):
        csl = slice(c * BS, (c + 1) * BS)
        lnz = np.log(res[c]["out_z"].astype(np.float64))     # [128, NZ]
        wmask = _TCOLS < ilen[csl][_UCOLS][:, None]          # [128, NZ]
        slz = np.zeros(BS)
        np.add.at(slz, _UCOLS.repeat(NZ),
                  (lnz * wmask).reshape(-1))
        ll = np.log(res[c]["out_cs"].astype(np.float64)).sum(axis=1)  # [BS]
        losses[csl] = -(ll + sum_gmax[csl] - slz)
    return np.float32(losses.mean())


# revision 16
# speedup vs baseline: 1.0067x; 1.0067x over previous
"""CTC loss on 8 Trainium2 cores.

Strategy (data-parallel over batch, B=64 -> 8 utterances/core):
  Device per core, two concurrent pipelines:
    - Streaming (ACT + HWDGE-DMA): acts [3200, 5000] f32 in 13 big tiles
      [128, 2, 5000]; ScalarE exp with accum_out -> Z per (t,u) row.
      ~64MB/core, memory-bound.
    - CTC DP on DVE only, layout [8 utterances (partitions), 101 ext states
      (free)]: shifts are free-axis offset views with zero guard columns;
      per step 3 DVE ops (add, add, dup-mul via broadcast+2-block write)
      in bf16 (2x DVE mode). Skip mask folded into a second emission table
      q1 = q0*mask; emissions q = exp(gtilde) precomputed host-side in
      bf16, streamed in chunks (the first four, small, ride the ACT HWDGE
      ring ahead of the acts tiles for a fast DP start; the rest use the
      sync-engine ring). Rescale by sum every 32 steps via
      scalar_tensor_tensor with a per-partition reciprocal; the STT accum
      also logs the rescale constants c into cs.
    - Freeze (t >= input_len) and final readout folded into one-hot q
      columns; final step's STT accum gives the end-state mass directly.
  Device outputs are raw Z [128, 25] and cs [8, 13]; the host does the
  tiny ln + masked reductions, corrections sum(gmax) - sum(logZ), mean.
  (tensor_tensor_reduce is avoided: it crashes this runtime; fp16 state
  underflows CTC alpha — bf16 is required.)
"""
import numpy as np
import ml_dtypes

import concourse.bass as bass
import concourse.bacc as bacc
import concourse.mybir as mybir
import concourse.tile as tile
from concourse.bass_utils import run_bass_kernel_spmd

T, B, V, L = 400, 64, 5000, 50
S = 2 * L + 1            # 101
NCORES = 8
BS = B // NCORES         # 8
ROWS = T * BS            # 3200
NT = 12                  # full [128, 2, V] tiles; + one [128, V] tail tile
NZ = 2 * NT + 1          # 25 Z slots
BOOST = np.float32(1.5)
K_RES = 32
NRES = T // K_RES        # 12 rescales; cs has NRES+1 slots
QW = 2 * S               # 202
QCOLS = (T + 1) * QW     # 81002
# q chunk boundaries in t: small first chunks for a fast DP start
CHB = [0, 5, 12, 22, 37, 57, 87, 127, 177, 227, 277, 327, 377, T + 1]
CHMAX = max(b - a for a, b in zip(CHB, CHB[1:]))
NEG = np.float32(-10000.0)
F32 = mybir.dt.float32
BF16 = mybir.dt.bfloat16
bf16 = ml_dtypes.bfloat16
AF = mybir.ActivationFunctionType
ALU = mybir.AluOpType


def _build_program():
    nc = bacc.Bacc(None, target_bir_lowering=False)
    acts = nc.dram_tensor("acts", [ROWS, V], F32, kind="ExternalInput")
    qq = nc.dram_tensor("qq", [BS, QCOLS], BF16, kind="ExternalInput")
    out_z = nc.dram_tensor("out_z", [128, NZ], F32, kind="ExternalOutput")
    out_cs = nc.dram_tensor("out_cs", [BS, NRES + 1], F32,
                            kind="ExternalOutput")

    with tile.TileContext(nc) as tc:
        with (
            tc.tile_pool(name="mp", bufs=1) as mp,
            tc.tile_pool(name="ap", bufs=2) as ap,
            tc.tile_pool(name="qp", bufs=5) as qp,
        ):
            # ---------------- persistent state ----------------
            ws = mp.tile([BS, 2 * S + 4], BF16)   # [g g a(101) g g b(101)]
            nc.vector.memset(ws[:], 0.0)
            t1 = mp.tile([BS, S], BF16)
            t2 = mp.tile([BS, S], BF16)
            cs = mp.tile([BS, NRES + 1], F32)
            rbuf = mp.tile([BS, 1], F32)
            zbuf = mp.tile([128, NZ], F32)

            def wsb():   # state write view: cols {2..102, 105..205}
                return ws[:].rearrange("p (b s) -> p b s", b=2)[:, :, 2:S + 2]

            def t2b():   # t2 broadcast over the two state blocks
                return t2[:].unsqueeze(1).broadcast_to((BS, 2, S))

            # ------------- streaming pipeline (ACT engine + ring) -------
            # First two (small) q chunks ride the ACT HWDGE ring ahead of
            # the 5MB acts tiles, so the DP starts within a few us.
            qts = {}
            for c in range(4):
                qte = qp.tile([BS, CHMAX * QW], BF16, tag="q")
                qts[c] = qte
                nc.scalar.dma_start(
                    qte[:, 0:(CHB[c + 1] - CHB[c]) * QW],
                    qq[:, CHB[c] * QW:CHB[c + 1] * QW])
            for k in range(NT + 1):
                at = ap.tile([128, 2 * V], F32, tag="acts")
                if k < NT:
                    src = acts[256 * k:256 * (k + 1), :].rearrange(
                        "(b p) v -> p b v", b=2)
                    nc.scalar.dma_start(at[:], src)
                    for bb in range(2):
                        blk = at[:, bb * V:(bb + 1) * V]
                        nc.scalar.activation(blk, blk, AF.Exp,
                                             accum_out=zbuf[:, 2 * k + bb:
                                                            2 * k + bb + 1])
                else:
                    nc.scalar.dma_start(at[:, 0:V], acts[256 * NT:ROWS, :])
                    blk = at[:, 0:V]
                    nc.scalar.activation(blk, blk, AF.Exp,
                                         accum_out=zbuf[:, 2 * NT:2 * NT + 1])
            nc.scalar.dma_start(out_z[:], zbuf[:])

            # ------------- DP pipeline (DVE + sync-ring DMA) ------------
            for c in range(len(CHB) - 1):
                t0, tend = CHB[c], CHB[c + 1]
                if c < 4:
                    qt = qts[c]
                else:
                    qt = qp.tile([BS, CHMAX * QW], BF16, tag="q")
                    nc.sync.dma_start(qt[:, 0:(tend - t0) * QW],
                                      qq[:, t0 * QW:tend * QW])
                trange = range(max(t0, 1), tend)
                if c == 0:
                    nc.vector.tensor_copy(
                        wsb(), qt[:, 0:QW].rearrange("p (b s) -> p b s", b=2))
                for t in trange:
                    base = (t - t0) * QW
                    w = min(2 * t + 2, S)   # active-state prefix band
                    q01 = qt[:, base:base + QW].rearrange(
                        "p (b s) -> p b s", b=2)[:, :, 0:w]
                    wsv = wsb()[:, :, 0:w]
                    t2v = t2[:, 0:w].unsqueeze(1).broadcast_to((BS, 2, w))
                    nc.vector.tensor_add(t1[:, 0:w], ws[:, 2:w + 2],
                                         ws[:, 1:w + 1])
                    nc.vector.tensor_add(t2[:, 0:w], t1[:, 0:w],
                                         ws[:, S + 2:S + 2 + w])
                    if t % K_RES == K_RES - 1:
                        j = t // K_RES
                        nc.vector.scalar_tensor_tensor(
                            wsv, t2v, 1.0, q01, ALU.mult, ALU.mult,
                            accum_out=cs[:, j:j + 1])
                        nc.vector.reciprocal(rbuf[:], cs[:, j:j + 1])
                    elif t % K_RES == 0:
                        nc.vector.scalar_tensor_tensor(
                            wsv, t2v, rbuf[:, 0:1], q01, ALU.mult, ALU.mult)
                    elif t == T:
                        nc.vector.scalar_tensor_tensor(
                            wsv, t2v, 1.0, q01, ALU.mult, ALU.mult,
                            accum_out=cs[:, NRES:NRES + 1])
                    else:
                        nc.vector.tensor_mul(wsv, t2v, q01)
            nc.sync.dma_start(out_cs[:], cs[:])
    nc.compile()
    return nc


_PROGRAM = None
_LAST_RESULTS = None


def _get_program():
    global _PROGRAM
    if _PROGRAM is None:
        _PROGRAM = _build_program()
    return _PROGRAM


def _host_prep(acts, ilen, labels, llen):
    """Per-core input maps plus host-side correction sums."""
    ext = np.zeros((B, S), np.int64)
    ext[:, 1::2] = labels
    m = np.zeros((B, S), np.float32)
    m[:, 2:] = ((ext[:, 2:] != 0) & (ext[:, 2:] != ext[:, :-2])).astype(
        np.float32)
    mtil = np.zeros((B, S), np.float32)
    mtil[:, :S - 2] = m[:, 2:]

    g = np.take_along_axis(acts, np.broadcast_to(ext[None], (T, B, S)), axis=2)
    gmax = g.max(axis=2).astype(np.float32) - BOOST          # [T, B]
    gt = (g - gmax[:, :, None]).astype(np.float32)           # [T, B, S]

    srange = np.arange(S)
    valid_s = srange[None, :] < (2 * llen + 1)[:, None]      # [B, S]
    gt = np.where(valid_s[None], gt, NEG)
    onehot = np.where(srange[None, :] == (2 * llen)[:, None],
                      np.float32(0.0), NEG)                  # [B, S]
    tmask = np.arange(T)[:, None] < ilen[None, :]            # [T, B]
    gt = np.where(tmask[:, :, None], gt, onehot[None])
    gt[0, :, 2:] = NEG
    gt_all = np.concatenate([gt, onehot[None]], axis=0)      # [T+1, B, S]

    q0 = np.exp(gt_all, dtype=np.float32)                    # [T+1, B, S]
    q1 = q0 * mtil[None]
    qarr = np.empty((B, T + 1, 2, S), np.float32)
    qarr[:, :, 0, :] = q0.transpose(1, 0, 2)
    qarr[:, :, 1, :] = q1.transpose(1, 0, 2)
    qq_full = qarr.reshape(B, QCOLS).astype(bf16)

    sum_gmax = (gmax.astype(np.float64) * tmask).sum(axis=0)  # [B]

    in_maps = []
    for c in range(NCORES):
        csl = slice(c * BS, (c + 1) * BS)
        acts_c = np.ascontiguousarray(
            acts[:, csl, :].reshape(ROWS, V).astype(np.float32))
        qq_c = np.ascontiguousarray(qq_full[csl])
        in_maps.append({"acts": acts_c, "qq": qq_c})
    return in_maps, sum_gmax


# z slot mapping: out_z[p, c] is Z for acts row r(p, c); t = r//8, u = p%8
_P = np.arange(128)
_RCOLS = np.empty((128, NZ), np.int64)
for _c in range(2 * NT):
    _RCOLS[:, _c] = 256 * (_c // 2) + 128 * (_c % 2) + _P
_RCOLS[:, 2 * NT] = 256 * NT + _P
_TCOLS = _RCOLS // BS          # [128, NZ]
_UCOLS = _P % BS               # [128]


def kernel(activations, input_lengths, labels, label_lengths):
    acts = np.asarray(activations, dtype=np.float32)
    ilen = np.asarray(input_lengths, dtype=np.int64)
    labs = np.asarray(labels, dtype=np.int64)
    llen = np.asarray(label_lengths, dtype=np.int64)

    in_maps, sum_gmax = _host_prep(acts, ilen, labs, llen)
    nc = _get_program()
    _r = run_bass_kernel_spmd(nc, in_maps, list(range(NCORES)))
    global _LAST_RESULTS
    _LAST_RESULTS = _r
    res = _r.results

    losses = np.zeros(B, np.float64)
    for c in range(NCORES):
        csl = slice(c * BS, (c + 1) * BS)
        lnz = np.log(res[c]["out_z"].astype(np.float64))     # [128, NZ]
        wmask = _TCOLS < ilen[csl][_UCOLS][:, None]          # [128, NZ]
        slz = np.zeros(BS)
        np.add.at(slz, _UCOLS.repeat(NZ),
                  (lnz * wmask).reshape(-1))
        ll = np.log(res[c]["out_cs"].astype(np.float64)).sum(axis=1)  # [BS]
        losses[csl] = -(ll + sum_gmax[csl] - slz)
    return np.float32(losses.mean())
